# revision 1
# baseline (speedup 1.0000x reference)
"""NT-Xent contrastive loss on 8 Trainium2 NeuronCores (Bass/Tile).

Strategy (no collectives -- measured ncfw latency floor ~85us makes the
all-gather hint design strictly worse):
  * Host pre-transposes embedded_data to embT [2048, 8192] (pure layout).
  * Slab cover: core c loads the 4 row-slabs S_c = {c, c+1, c+2, c+4} (mod 8)
    of emb (32 MiB/core). Every slab PAIR meets on some core (Z8 difference
    cover: slot-pairs at differences 1,2,3,4), so each of the 36 distinct
    1024x1024 blocks of the 8192x8192 similarity matrix is computed once
    globally; block (i,j) yields exp-row-sums for slab i (ACT accum) AND
    exp-col-sums for slab j (ones-matmul), exploiting sim symmetry.
  * Per core, uniform SPMD program: head matmul out_headT = W.T @ embT_slab
    (fp32r, 1 cyc/row), L2 normalize via ones-matmul normsq + Sqrt +
    reciprocal + K=1 broadcast matmul, then 5 sim blocks (diag + 4 pairs):
    psum [128,1024] fp32 -> ACT exp(10*x) with fused row-sum accum ->
    f32r exp tile -> ones-matmul col-sums. Diagonal exp values extracted
    exactly via a shifted-identity mask (mult+reduce) and subtracted on host.
  * pos term: elementwise product of slabs c and c+4 + ones-matmul -> the
    positive-pair similarities; log(pos) = 10*possim exactly (no exp needed).
  * Host (fp64): sums partial row/col contributions, subtracts diag,
    loss = -mean(10*possim - log(neg)).
"""
import numpy as np

SLOTS = [(c, (c + 1) % 8, (c + 2) % 8, (c + 4) % 8) for c in range(8)]
# blocks in local slot coords: (stationary, moving). B0 = diag.
BLOCKS = [(0, 0), (0, 1), (0, 2), (1, 3), (0, 3)]

_CACHE = {}


def _build():
    if "nc" in _CACHE:
        return _CACHE["nc"]
    import concourse.bacc as bacc
    import concourse.tile as tile
    import concourse.mybir as mybir

    F32, F32R = mybir.dt.float32, mybir.dt.float32r
    AF = mybir.ActivationFunctionType
    ALU = mybir.AluOpType

    nc = bacc.Bacc("TRN2", num_devices=8, debug=False)
    a_emb = nc.dram_tensor("embT", [2048, 4096], F32, kind="ExternalInput").ap()
    a_W = nc.dram_tensor("W", [2048, 256], F32, kind="ExternalInput").ap()
    a_b = nc.dram_tensor("b", [256], F32, kind="ExternalInput").ap()
    a_ones = nc.dram_tensor("ones", [128, 128], F32, kind="ExternalInput").ap()
    a_mask = nc.dram_tensor("mask", [128, 2048], F32, kind="ExternalInput").ap()
    o_rp = nc.dram_tensor("rowpart", [5, 1024], F32, kind="ExternalOutput").ap()
    o_cp = nc.dram_tensor("colpart", [4, 1024], F32, kind="ExternalOutput").ap()
    o_dg = nc.dram_tensor("diagexp", [1, 1024], F32, kind="ExternalOutput").ap()
    o_ps = nc.dram_tensor("possim", [1, 1024], F32, kind="ExternalOutput").ap()

    with tile.TileContext(nc) as tc:
        with tc.tile_pool(name="sb", bufs=1) as sb, \
             tc.tile_pool(name="emb", bufs=10) as embp, \
             tc.tile_pool(name="work", bufs=2) as wk, \
             tc.tile_pool(name="expp", bufs=3) as expp, \
             tc.tile_pool(name="headp", bufs=1, space="PSUM") as headp, \
             tc.tile_pool(name="simp", bufs=2, space="PSUM") as simp, \
             tc.tile_pool(name="csp", bufs=2, space="PSUM") as csp:

            t_W = sb.tile([128, 16, 256], F32R, name="t_W")
            nc.sync.dma_start(t_W[:], a_W.bitcast(F32R).rearrange("(kc p) d -> p kc d", p=128))
            t_b = sb.tile([128, 2], F32, name="t_b")
            nc.sync.dma_start(t_b[:], a_b.rearrange("(dh p) -> p dh", p=128))
            ones_col = sb.tile([128, 1], F32R, name="ones_col")
            nc.sync.dma_start(ones_col[:], a_ones.bitcast(F32R)[:, 0:1])
            ones_row = sb.tile([1, 128], F32, name="ones_row")
            nc.sync.dma_start(ones_row[:], a_ones[0:1, :])
            t_mask = sb.tile([128, 2048], F32, name="t_mask")
            nc.sync.dma_start(t_mask[:], a_mask[:])

            # staging accumulators
            rp_st = sb.tile([128, 5, 8], F32, name="rp_st")
            dg_st = sb.tile([128, 8], F32, name="dg_st")
            cp_st = sb.tile([1, 4096], F32, name="cp_st")
            ps_st = sb.tile([1, 1024], F32, name="ps_st")

            t_on = [sb.tile([128, 2, 1024], F32R, name=f"t_on{k}") for k in range(4)]

            def stage_a(k):
                t_h = wk.tile([128, 2, 1024], F32, name="t_h", tag="th")
                for h in range(2):
                    tes = []
                    for g in range(8):
                        t_e = embp.tile([128, 2, 512], F32R, name="t_e", tag="emb")
                        src = a_emb.bitcast(F32R)[256 * g:256 * (g + 1),
                                                  1024 * k + 512 * h:1024 * k + 512 * (h + 1)]
                        nc.sync.dma_start(t_e[:], src.rearrange("(c p) r -> p c r", p=128))
                        tes.append(t_e)
                    p_h = headp.tile([128, 2, 512], F32, name="p_h", tag="head")
                    for g in range(8):
                        for cc in range(2):
                            kk = 2 * g + cc
                            for dh in range(2):
                                nc.tensor.matmul(
                                    p_h[:, dh, :],
                                    t_W[:, kk, dh * 128:(dh + 1) * 128],
                                    tes[g][:, cc, :],
                                    start=(kk == 0), stop=(kk == 15),
                                )
                    for dh in range(2):
                        nc.vector.tensor_scalar_add(
                            t_h[:, dh, 512 * h:512 * (h + 1)], p_h[:, dh, :],
                            t_b[:, dh:dh + 1])
                t_sq = wk.tile([128, 2, 1024], F32R, name="t_sq", tag="sq")
                nc.vector.tensor_tensor(t_sq[:], t_h[:], t_h[:], ALU.mult)
                p_ns = [csp.tile([1, 512], F32, name=f"p_ns{nb}", tag="cs") for nb in range(2)]
                for nb in range(2):
                    for dh in range(2):
                        nc.tensor.matmul(p_ns[nb][:], ones_col[:],
                                         t_sq[:, dh, 512 * nb:512 * (nb + 1)],
                                         start=(dh == 0), stop=(dh == 1))
                t_nrm = wk.tile([1, 1024], F32, name="t_nrm", tag="nrm")
                for nb in range(2):
                    nc.scalar.activation(t_nrm[:, 512 * nb:512 * (nb + 1)], p_ns[nb][:], AF.Sqrt)
                t_ri = wk.tile([1, 1024], F32, name="t_ri", tag="ri")
                nc.vector.reciprocal(t_ri[:], t_nrm[:])
                p_bc = headp.tile([128, 2, 512], F32, name="p_bc", tag="head")
                for nb in range(2):
                    nc.tensor.matmul(p_bc[:, nb, :], ones_row[:],
                                     t_ri[:, 512 * nb:512 * (nb + 1)], start=True, stop=True)
                bc_flat = p_bc[:].rearrange("p a b -> p (a b)")
                for dh in range(2):
                    nc.vector.tensor_tensor(t_on[k][:, dh, :], t_h[:, dh, :], bc_flat, ALU.mult)

            def block(bslot, a, bm):
                p_cs = None
                if bslot > 0:
                    p_cs = [csp.tile([1, 512], F32, name=f"p_cs{bslot}_{nb}", tag="cs")
                            for nb in range(2)]
                for mb in range(8):
                    p_sim = simp.tile([128, 1024], F32, name="p_sim", tag="sim")
                    for dh in range(2):
                        for nb in range(2):
                            nc.tensor.matmul(
                                p_sim[:, 512 * nb:512 * (nb + 1)],
                                t_on[a][:, dh, 128 * mb:128 * (mb + 1)],
                                t_on[bm][:, dh, 512 * nb:512 * (nb + 1)],
                                start=(dh == 0), stop=(dh == 1))
                    t_exp = expp.tile([128, 1024], F32R, name="t_exp", tag="exp")
                    nc.scalar.activation(t_exp[:], p_sim[:], AF.Exp, scale=10.0,
                                         accum_out=rp_st[:, bslot, mb:mb + 1])
                    if bslot > 0:
                        for nb in range(2):
                            nc.tensor.matmul(p_cs[nb][:], ones_col[:],
                                             t_exp[:, 512 * nb:512 * (nb + 1)],
                                             start=(mb == 0), stop=(mb == 7))
                    else:
                        t_sc = expp.tile([128, 1024], F32, name="t_sc", tag="sc")
                        nc.vector.tensor_tensor(
                            t_sc[:], t_exp[:].bitcast(F32),
                            t_mask[:, 1024 - 128 * mb:2048 - 128 * mb], ALU.mult)
                        nc.vector.tensor_reduce(dg_st[:, mb:mb + 1], t_sc[:],
                                                mybir.AxisListType.X, ALU.add)
                if bslot > 0:
                    for nb in range(2):
                        nc.vector.tensor_copy(
                            cp_st[0:1, 1024 * (bslot - 1) + 512 * nb:
                                  1024 * (bslot - 1) + 512 * (nb + 1)], p_cs[nb][:])

            stage_a(0)
            block(0, 0, 0)
            stage_a(1)
            block(1, 0, 1)
            stage_a(2)
            block(2, 0, 2)
            stage_a(3)
            block(3, 1, 3)
            block(4, 0, 3)

            # pos: elementwise product slabs slot0 x slot3, column sums over d
            t_pp = wk.tile([128, 2, 1024], F32R, name="t_pp", tag="sq")
            for dh in range(2):
                nc.vector.tensor_tensor(t_pp[:, dh, :], t_on[0][:, dh, :].bitcast(F32),
                                        t_on[3][:, dh, :].bitcast(F32), ALU.mult)
            p_ps = [csp.tile([1, 512], F32, name=f"p_ps{nb}", tag="cs") for nb in range(2)]
            for nb in range(2):
                for dh in range(2):
                    nc.tensor.matmul(p_ps[nb][:], ones_col[:],
                                     t_pp[:, dh, 512 * nb:512 * (nb + 1)],
                                     start=(dh == 0), stop=(dh == 1))
                nc.vector.tensor_copy(ps_st[0:1, 512 * nb:512 * (nb + 1)], p_ps[nb][:])

            # final DMAs
            for bslot in range(5):
                nc.sync.dma_start(
                    o_rp[bslot:bslot + 1, :].rearrange("o (m p) -> p (o m)", p=128),
                    rp_st[:, bslot, :])
            nc.sync.dma_start(o_dg.rearrange("o (m p) -> p (o m)", p=128), dg_st[:])
            nc.sync.dma_start(o_cp.rearrange("a r -> (a r)")[None, :], cp_st[:])
            nc.sync.dma_start(o_ps[:], ps_st[:])

    nc.compile()
    _CACHE["nc"] = nc
    return nc


def _host_inputs(embedded_data, W, b):
    embT = np.ascontiguousarray(np.asarray(embedded_data, dtype=np.float32).T)
    W = np.asarray(W, dtype=np.float32)
    b = np.asarray(b, dtype=np.float32)
    mask = np.zeros((128, 2048), np.float32)
    mask[np.arange(128), np.arange(128) + 1024] = 1.0
    ones = np.ones((128, 128), np.float32)
    in_maps = []
    for c in range(8):
        cols = np.concatenate(
            [embT[:, 1024 * s:1024 * (s + 1)] for s in SLOTS[c]], axis=1)
        in_maps.append({"embT": np.ascontiguousarray(cols), "W": W, "b": b,
                        "ones": ones, "mask": mask})
    return in_maps


def _combine(results):
    neg = np.zeros(8192, np.float64)
    pos = np.zeros(8192, np.float64)
    for c in range(8):
        S = SLOTS[c]
        rp = results[c]["rowpart"].astype(np.float64)
        cp = results[c]["colpart"].astype(np.float64)
        dg = results[c]["diagexp"].astype(np.float64).ravel()
        sl = [np.s_[1024 * s:1024 * (s + 1)] for s in S]
        neg[sl[0]] += rp[0] - dg          # diag block, self-sim removed
        neg[sl[0]] += rp[1]; neg[sl[1]] += cp[0]   # B1 (0,1)
        neg[sl[0]] += rp[2]; neg[sl[2]] += cp[1]   # B2 (0,2)
        neg[sl[1]] += rp[3]; neg[sl[3]] += cp[2]   # B3 (1,3)
        if c < 4:                                   # B4 (0,3) dedup: cores 0-3
            neg[sl[0]] += rp[4]; neg[sl[3]] += cp[3]
            ps = results[c]["possim"].astype(np.float64).ravel()
            pos[sl[0]] = ps
            pos[1024 * S[3]:1024 * (S[3] + 1)] = ps
    loss = -np.mean(10.0 * pos - np.log(neg))
    return np.float32(loss)


def run(embedded_data, W, b, trace=False):
    from concourse import bass_utils
    nc = _build()
    in_maps = _host_inputs(embedded_data, W, b)
    res = bass_utils.run_bass_kernel_spmd(nc, in_maps, core_ids=list(range(8)),
                                          trace=trace)
    return _combine(res.results), res


def kernel(embedded_data, W, b):
    loss, _ = run(embedded_data, W, b, trace=False)
    return np.asarray(loss, dtype=np.float32)



# revision 11
# speedup vs baseline: 2.0153x; 2.0153x over previous
"""NT-Xent contrastive loss on 8 Trainium2 NeuronCores (Bass/Tile), v2.

Strategy (no collectives; slab difference-cover as v1, but fp8 everywhere):
  * Host casts embT and W (x64) to fp8 e4m3. Uniform input scaling is exact:
    the L2 normalize cancels any scalar factor, and the normalize multiplier
    is computed as r = exp(-0.5*ln(normsq) + ln 16) so the normalized
    vectors come out scaled x16 (fp8 sweet spot) regardless of input scale.
    Ln/Exp/Copy share one scalar-engine activation table -> zero table swaps.
  * All big matmuls run fp8 DoubleRow (2 k-subtiles of 128 per pass, 0.5
    cyc/row): head (K=2048 = 8 DR passes), sim blocks (K=256 = 1 DR pass),
    and the colsum ones-matmuls (pairs of exp tiles as the 2 k-subtiles).
  * Off-diagonal exp tiles are written fp8 as exp(10*s - 3) (range safe:
    |s|<=0.45 off-diag), rowsums exact via ACT accum (fp32), colsums from
    the fp8 tiles via DR ones-matmul; host rescales by e^3.
  * Diag-block exp tiles (bf16, contain s=1) are DMA'd to HBM; the host
    extracts the diagonal exp values to subtract self-similarity exactly.
  * pos term: elementwise product of slabs slot0*slot3 + ones-matmul.
  * PSUM budget 16KB/partition: head rotation 2x[128,512] + sim
    double-buffer 2x[128,1024] + smalls rotation 2x[128,512] (normsq rows,
    colsum tiles with results at partition offsets 0/32/64/96).
"""
import math
import numpy as np

SLOTS = [(c, (c + 1) % 8, (c + 2) % 8, (c + 4) % 8) for c in range(8)]
# blocks in local slot coords: (stationary, moving). B0 = diag.
BLOCKS = [(0, 0), (0, 1), (0, 2), (1, 3), (0, 3)]

W_SCALE = 64.0
V_SCALE = 16.0  # normalized vectors scaled x16 into fp8
EXP_BIAS = -3.0  # exp(10*s + EXP_BIAS) keeps off-diag exps in fp8 range

_CACHE = {}


def _build():
    if "nc" in _CACHE:
        return _CACHE["nc"]
    import concourse.bacc as bacc
    import concourse.tile as tile
    import concourse.mybir as mybir

    F32, BF16, F8 = mybir.dt.float32, mybir.dt.bfloat16, mybir.dt.float8e4
    AF = mybir.ActivationFunctionType
    ALU = mybir.AluOpType
    DR = mybir.MatmulPerfMode.DoubleRow

    nc = bacc.Bacc("TRN2", num_devices=8, debug=False)
    a_emb = nc.dram_tensor("embQ", [2048, 4096], F8, kind="ExternalInput").ap()
    a_W = nc.dram_tensor("Wq", [2048, 256], F8, kind="ExternalInput").ap()
    a_b = nc.dram_tensor("b64", [256], F32, kind="ExternalInput").ap()
    a_ones8 = nc.dram_tensor("ones8", [128, 64], F8, kind="ExternalInput").ap()
    a_ones16 = nc.dram_tensor("ones16", [128, 1], BF16, kind="ExternalInput").ap()
    o_rp = nc.dram_tensor("rowpart", [5, 1024], F32, kind="ExternalOutput").ap()
    o_cp = nc.dram_tensor("colpart", [1, 8192], F32, kind="ExternalOutput").ap()
    o_dx = nc.dram_tensor("dexp", [8, 128, 1024], BF16, kind="ExternalOutput").ap()
    o_ps = nc.dram_tensor("possim", [1, 1024], F32, kind="ExternalOutput").ap()

    with tile.TileContext(nc) as tc:
        with tc.tile_pool(name="sb", bufs=1) as sb, \
             tc.tile_pool(name="emb", bufs=2) as embp, \
             tc.tile_pool(name="wk", bufs=2) as wk, \
             tc.tile_pool(name="expp", bufs=2) as expp, \
             tc.tile_pool(name="hp", bufs=2, space="PSUM") as hp, \
             tc.tile_pool(name="sp", bufs=2, space="PSUM") as spp, \
             tc.tile_pool(name="smp", bufs=2, space="PSUM") as smp:

            t_W = sb.tile([128, 16, 256], F8, name="t_W")
            nc.sync.dma_start(t_W[:], a_W.rearrange("(kc p) d -> p kc d", p=128))
            t_b = sb.tile([128, 2], F32, name="t_b")
            nc.sync.dma_start(t_b[:], a_b.rearrange("(dh p) -> p dh", p=128))
            ones8 = sb.tile([128, 2, 32], F8, name="ones8")
            nc.sync.dma_start(ones8[:],
                              a_ones8.rearrange("p (c u) -> p c u", u=32))
            ones16 = sb.tile([128, 1], BF16, name="ones16")
            nc.sync.dma_start(ones16[:], a_ones16[:])
            t_cb = sb.tile([128, 2], F32, name="t_cb")
            nc.gpsimd.memset(t_cb[:, 0:1], math.log(V_SCALE))
            nc.gpsimd.memset(t_cb[:, 1:2], EXP_BIAS)

            # staging
            rp_st = sb.tile([128, 5, 8], F32, name="rp_st")
            cs_st = sb.tile([1, 8192], F32, name="cs_st")
            ps_st = sb.tile([1, 1024], F32, name="ps_st")

            t_on = [sb.tile([128, 2, 1024], F8, name=f"t_on{k}") for k in range(4)]

            def stage_a(k):
                t_e = embp.tile([128, 16, 1024], F8, name="t_e", tag="emb")
                nc.sync.dma_start(
                    t_e[:],
                    a_emb[:, 1024 * k:1024 * (k + 1)].rearrange(
                        "(c p) r -> p c r", p=128))
                t_h = wk.tile([128, 2, 1024], BF16, name="t_h", tag="th")
                for h in range(2):
                    for dh in range(2):
                        p_h = hp.tile([128, 512], F32, name="p_h", tag="hp")
                        for g in range(8):
                            nc.tensor.matmul(
                                p_h[:],
                                t_W[:, 2 * g:2 * g + 2, 128 * dh:128 * (dh + 1)],
                                t_e[:, 2 * g:2 * g + 2, 512 * h:512 * (h + 1)],
                                start=(g == 0), stop=(g == 7), perf_mode=DR)
                        nc.vector.tensor_scalar_add(
                            t_h[:, dh, 512 * h:512 * (h + 1)], p_h[:],
                            t_b[:, dh:dh + 1])
                t_sq = wk.tile([128, 2, 1024], BF16, name="t_sq", tag="sq")
                nc.vector.tensor_tensor(t_sq[:], t_h[:], t_h[:], ALU.mult)
                t_ln = wk.tile([1, 1024], F32, name="t_ln", tag="ln")
                for nb in range(2):
                    p_ns = smp.tile([128, 512], F32, name="p_ns", tag="sm")
                    for dh in range(2):
                        nc.tensor.matmul(p_ns[0:1, :], ones16[:],
                                         t_sq[:, dh, 512 * nb:512 * (nb + 1)],
                                         start=(dh == 0), stop=(dh == 1))
                    nc.scalar.activation(t_ln[:, 512 * nb:512 * (nb + 1)],
                                         p_ns[0:1, :], AF.Ln)
                t_r = wk.tile([1, 1024], F32, name="t_r", tag="r")
                nc.scalar.activation(t_r[:], t_ln[:], AF.Exp,
                                     scale=-0.5, bias=t_cb[0:1, 0:1])
                t_bc = wk.tile([128, 1024], F32, name="t_bc", tag="bc")
                nc.gpsimd.partition_broadcast(t_bc[:], t_r[:])
                for dh in range(2):
                    nc.vector.tensor_tensor(t_on[k][:, dh, :], t_h[:, dh, :],
                                            t_bc[:], ALU.mult)

            def block(bslot, a, bm):
                for mb in range(8):
                    p_sim = spp.tile([128, 1024], F32, name="p_sim", tag="sp")
                    for nb in range(2):
                        nc.tensor.matmul(
                            p_sim[:, 512 * nb:512 * (nb + 1)],
                            t_on[a][:, :, 128 * mb:128 * (mb + 1)],
                            t_on[bm][:, :, 512 * nb:512 * (nb + 1)],
                            start=True, stop=True, perf_mode=DR)
                    if bslot == 0:
                        t_e0 = expp.tile([128, 1024], BF16, name="t_e0", tag="e0")
                        nc.scalar.activation(
                            t_e0[:], p_sim[:], AF.Exp,
                            scale=10.0 / (V_SCALE * V_SCALE),
                            bias=t_cb[:, 1:2],
                            accum_out=rp_st[:, 0, mb:mb + 1])
                        nc.sync.dma_start(o_dx[mb], t_e0[:])
                    else:
                        if mb % 2 == 0:
                            t_ex = expp.tile([128, 2, 1024], F8, name="t_ex",
                                             tag="te")
                        nc.scalar.activation(
                            t_ex[:, mb % 2, :], p_sim[:], AF.Exp,
                            scale=10.0 / (V_SCALE * V_SCALE),
                            bias=t_cb[:, 1:2],
                            accum_out=rp_st[:, bslot, mb:mb + 1])
                        if mb % 2 == 1:
                            pair = mb // 2  # 0..3
                            if pair % 2 == 0:
                                p_cs = [smp.tile([128, 512], F32,
                                                 name=f"p_cs{nb}", tag="sm")
                                        for nb in range(2)]
                            for nb in range(2):
                                nc.tensor.matmul(
                                    p_cs[nb][0:32, :],
                                    ones8[:],
                                    t_ex[:, :, 512 * nb:512 * (nb + 1)],
                                    start=(pair % 2 == 0),
                                    stop=(pair % 2 == 1), perf_mode=DR)
                            if pair % 2 == 1:
                                # 2 results [1,512] -> cs_st free-dim slots
                                grp = 2 * (2 * (bslot - 1) + pair // 2)
                                for nb in range(2):
                                    o = 512 * (grp + nb)
                                    nc.vector.tensor_copy(
                                        cs_st[0:1, o:o + 512],
                                        p_cs[nb][0:1, :])

            stage_a(0)
            stage_a(1)
            block(0, 0, 0)
            stage_a(2)
            block(1, 0, 1)
            stage_a(3)
            block(2, 0, 2)
            block(3, 1, 3)
            block(4, 0, 3)

            # pos: elementwise product slabs slot0 x slot3, colsum over feats
            t_pp = wk.tile([128, 2, 1024], BF16, name="t_pp", tag="sq")
            nc.vector.tensor_tensor(
                t_pp[:].rearrange("p a b -> p (a b)"),
                t_on[0][:].rearrange("p a b -> p (a b)"),
                t_on[3][:].rearrange("p a b -> p (a b)"), ALU.mult)
            for nb in range(2):
                p_ps = smp.tile([128, 512], F32, name="p_ps", tag="sm")
                for dh in range(2):
                    nc.tensor.matmul(p_ps[0:1, :], ones16[:],
                                     t_pp[:, dh, 512 * nb:512 * (nb + 1)],
                                     start=(dh == 0), stop=(dh == 1))
                nc.vector.tensor_copy(ps_st[0:1, 512 * nb:512 * (nb + 1)],
                                      p_ps[0:1, :])

            # final DMAs
            for bslot in range(5):
                nc.sync.dma_start(
                    o_rp[bslot:bslot + 1, :].rearrange("o (m p) -> p (o m)", p=128),
                    rp_st[:, bslot, :])
            nc.sync.dma_start(o_cp[:], cs_st[:])
            nc.sync.dma_start(o_ps[:], ps_st[:])

    nc.compile()
    _CACHE["nc"] = nc
    return nc


def _host_inputs(embedded_data, W, b):
    import ml_dtypes
    f8 = ml_dtypes.float8_e4m3
    embT = np.asarray(embedded_data, dtype=np.float32).T
    embQ = embT.astype(f8)
    Wq = (np.asarray(W, dtype=np.float32) * W_SCALE).astype(f8)
    b64 = np.asarray(b, dtype=np.float32) * W_SCALE
    ones8 = np.ones((128, 64), f8)
    ones16 = np.ones((128, 1), ml_dtypes.bfloat16)
    in_maps = []
    for c in range(8):
        cols = np.concatenate(
            [embQ[:, 1024 * s:1024 * (s + 1)] for s in SLOTS[c]], axis=1)
        in_maps.append({"embQ": np.ascontiguousarray(cols), "Wq": Wq,
                        "b64": b64, "ones8": ones8, "ones16": ones16})
    return in_maps


def _combine(results):
    e3 = math.exp(-EXP_BIAS)  # rescale exp(10s-3) -> exp(10s)
    neg = np.zeros(8192, np.float64)
    pos = np.zeros(8192, np.float64)
    idx = np.arange(1024)
    mb_of = idx // 128
    p_of = idx % 128
    for c in range(8):
        S = SLOTS[c]
        rp = results[c]["rowpart"].astype(np.float64)
        dx = results[c]["dexp"].astype(np.float64)
        cp = results[c]["colpart"].astype(np.float64).reshape(16, 512)
        # diag exp values: sample i (=128*mb+p) at dexp[mb, p, 128*mb+p]
        dg = dx[mb_of, p_of, idx]
        # colsums: cs_st partition group g=2*(2*(B-1)+H)+nb; each entry is
        # the colsum over a half-block (pairs 2H,2H+1) for cols nb*512+[0,512)
        csum = np.zeros((4, 1024), np.float64)
        for B in range(4):
            for H in range(2):
                for nb in range(2):
                    g = 2 * (2 * B + H) + nb
                    csum[B, nb * 512:(nb + 1) * 512] += cp[g]
        sl = [np.s_[1024 * s:1024 * (s + 1)] for s in S]
        neg[sl[0]] += e3 * (rp[0] - dg)            # diag block, self-sim removed
        neg[sl[0]] += e3 * rp[1]; neg[sl[1]] += e3 * csum[0]   # B1 (0,1)
        neg[sl[0]] += e3 * rp[2]; neg[sl[2]] += e3 * csum[1]   # B2 (0,2)
        neg[sl[1]] += e3 * rp[3]; neg[sl[3]] += e3 * csum[2]   # B3 (1,3)
        if c < 4:                                   # B4 (0,3) dedup: cores 0-3
            neg[sl[0]] += e3 * rp[4]; neg[sl[3]] += e3 * csum[3]
            ps = results[c]["possim"].astype(np.float64).ravel()
            ps = ps / (V_SCALE * V_SCALE)
            pos[sl[0]] = ps
            pos[1024 * S[3]:1024 * (S[3] + 1)] = ps
    loss = -np.mean(10.0 * pos - np.log(neg))
    return np.float32(loss)


def run(embedded_data, W, b, trace=False):
    from concourse import bass_utils
    nc = _build()
    in_maps = _host_inputs(embedded_data, W, b)
    res = bass_utils.run_bass_kernel_spmd(nc, in_maps, core_ids=list(range(8)),
                                          trace=trace)
    return _combine(res.results), res


def kernel(embedded_data, W, b):
    loss, _ = run(embedded_data, W, b, trace=False)
    return np.asarray(loss, dtype=np.float32)
